# revision 48
# baseline (speedup 1.0000x reference)
"""Trainium2 Bass kernel for nn_CrossAttentionFusion.

Math notes (exact simplifications of the reference):
  - MultiheadAttention with a single key/query position: softmax over one
    element == 1.0 exactly, so attn maps are constant ones and the q/k
    projections are dead code. _mha1(q,k,v,...) == (v @ wv.T + bv) @ wo.T + bo.
  - Therefore:
      t   = text @ tp_w.T + tp_b
      n   = num  @ np_w.T + np_b
      x1  = n + (t @ a1_wv.T + a1_bv) @ a1_wo.T + a1_bo ; n2t = LN(x1)*n1_g+n1_b
      x2  = t + (n @ a2_wv.T + a2_bv) @ a2_wo.T + a2_bo ; t2n = LN(x2)*n2_g+n2_b
      comb = [n2t, t2n]
      gate = sigmoid(comb @ g_w.T + g_b)
      h    = gelu(LN(comb @ m1_w.T + m1_b)*ln1_g+ln1_b)      (exact/erf gelu)
      fused= gelu(LN(h @ m2_w.T + m2_b)*ln2_g+ln2_b)
      attn outputs = ones((B,1,1))

Device strategy (8 cores, pure data parallel over batch):
  - Per core: 2048 rows, processed in 4 chunks of 512 rows.
  - Activations live features-major [feat_part, rows] for the matmul chain
    (weights stationary as lhsT). Layers feeding a LayerNorm are computed
    rows-major [row_part, feat] instead (activations stationary as lhsT),
    with residual + bias accumulated directly in PSUM (bias via rank-1
    ones-matmul, residual via transpose-matmuls accumulating into the bank).
  - LN: bn_stats/bn_aggr on DVE (free-dim reduce), rsqrt via DVE Newton
    iteration (bit-trick seed), apply via fused tensor_scalar. Back to
    features-major via PE transposes; gamma/beta folded into the ACT copyback.
  - gelu computed via erf so every ACT function used (identity/sigmoid/erf)
    lives in the single `sigmoid_and_others` table set -> one table load.
  - Matmuls run in float32r (full-rate fp32 PE mode), fp32 accumulate.
"""

import os
import sys

import numpy as np

for _p in ("/opt/trn_rl_repo", "/root/.axon_site/_ro/trn_rl_repo"):
    if os.path.isdir(_p) and _p not in sys.path:
        sys.path.insert(0, _p)

B = 16384
NCORES = 8
BC = B // NCORES          # rows per core (2048)
CH = 512                  # rows per chunk
NCH = BC // CH            # chunks per core (4)
P = 128
RT = CH // P              # row-tiles per chunk (4)
TD = 256
FD = 512
H1 = 512
H2 = 256
EPS = 1e-5
INV_SQRT2 = 0.7071067811865476

_CACHE = {}
STAGES = []  # (instruction-counter, label) marks from the last build


def _build_nc(ln1_identity: bool, ln2_identity: bool):
    from contextlib import ExitStack

    import concourse.bass as bass
    import concourse.tile as tile
    from concourse import bacc, mybir

    F32 = mybir.dt.float32
    F32R = mybir.dt.float32r
    I32 = mybir.dt.int32
    Alu = mybir.AluOpType
    Act = mybir.ActivationFunctionType

    nc = bacc.Bacc(
        "TRN2", target_bir_lowering=False, debug=False, enable_asserts=False
    )

    # ---- DRAM tensors -------------------------------------------------
    # float32r (same bits as fp32) for everything consumed by the PE so the
    # full-rate fp32r matmul path type-checks end to end.
    def din(name, shape, dt=F32):
        return nc.dram_tensor(name, list(shape), dt, kind="ExternalInput").ap()

    xt = din("xt", (TD, BC), F32R)    # text_emb shard, transposed
    xn = din("xn", (TD, BC), F32R)
    wtp = din("wtp", (TD, FD), F32R)  # tp_w.T
    wnp = din("wnp", (TD, FD), F32R)
    wv1 = din("wv1", (FD, FD), F32R)  # a1_wv.T
    wo1 = din("wo1", (FD, FD), F32R)
    wv2 = din("wv2", (FD, FD), F32R)
    wo2 = din("wo2", (FD, FD), F32R)
    wg = din("wg", (2 * FD, FD), F32R)      # g_w.T
    wm1 = din("wm1", (2 * FD, H1), F32R)    # m1_w.T
    wm2 = din("wm2", (H1, H2), F32R)        # m2_w.T
    ident = din("ident", (P, P), F32R)      # identity for PE transposes
    onesr = din("onesr", (1, P), F32R)      # ones row for rank-1 bias matmuls
    # All per-partition biases/gammas batched into one tensor: [12, 4, 128]
    # rows: btp,bnp,bv1,bv2,gm1,bt1,gm2,bt2,lg1,lb1,lg1s,lb1s
    bcat = din("bcat", (12, FD // P, P))
    # All rank-1 matmul bias rows batched: bc1|bc2|bg|bm1|bm2 -> [1, 2304]
    rcat = din("rcat", (1, 4 * FD + H2), F32R)
    lg2 = din("lg2", (H2,))
    lb2 = din("lb2", (H2,))

    fused = nc.dram_tensor("fused", [BC, H2], F32, kind="ExternalOutput").ap()
    gate = nc.dram_tensor("gate", [BC, FD], F32, kind="ExternalOutput").ap()

    xt3 = xt.rearrange("(s p) r -> p s r", p=P)     # [128, 2, BC]
    xn3 = xn.rearrange("(s p) r -> p s r", p=P)
    fused4 = fused.rearrange("(c t p) f -> p c t f", c=NCH, t=RT, p=P)
    gate4 = gate.rearrange("(c t p) f -> p c t f", c=NCH, t=RT, p=P)

    with tile.TileContext(nc) as tc, ExitStack() as ctx:
        singles = ctx.enter_context(tc.tile_pool(name="singles", bufs=1))
        iop = ctx.enter_context(tc.tile_pool(name="iop", bufs=2))
        acts = ctx.enter_context(tc.tile_pool(name="acts", bufs=1))
        zep = ctx.enter_context(tc.tile_pool(name="zep", bufs=2))
        outp = ctx.enter_context(tc.tile_pool(name="outp", bufs=1))
        stats = ctx.enter_context(tc.tile_pool(name="stats", bufs=4))
        ps = ctx.enter_context(tc.tile_pool(name="ps", bufs=6, space="PSUM"))
        psA = ctx.enter_context(tc.tile_pool(name="psA", bufs=2, space="PSUM"))

        # ---- one-time constants / weights --------------------------------
        # chunk-0 inputs first so compute can start immediately
        pref = []

        def prefetch(c):
            cs = slice(c * CH, (c + 1) * CH)
            xtc = iop.tile([P, 2, CH], F32R, tag="xtc", name="xtc")
            nc.sync.dma_start(xtc[:], xt3[:, :, cs])
            xnc = iop.tile([P, 2, CH], F32R, tag="xnc", name="xnc")
            nc.sync.dma_start(xnc[:], xn3[:, :, cs])
            pref.append((xtc, xnc))

        prefetch(0)

        def wload(nm, ap, ksub, n, engine=None):
            # Big weights ride the gpsimd (SWDGE) queue so per-chunk input
            # DMAs on the sync (HWDGE) queue aren't serialized behind them.
            t = singles.tile([P, ksub, n], F32R, tag=f"w_{nm}")
            eng = engine if engine is not None else nc.gpsimd
            eng.dma_start(t[:], ap.rearrange("(s p) f -> p s f", p=P))
            return t

        s_wtp = wload("wtp", wtp, 2, FD, engine=nc.sync)
        s_wnp = wload("wnp", wnp, 2, FD, engine=nc.sync)
        # batched small constants (one DMA each)
        s_bcat = singles.tile([P, 12, FD // P], F32, tag="bcat")
        nc.sync.dma_start(s_bcat[:], bcat.rearrange("b s p -> p b s"))
        (s_btp, s_bnp, s_bv1, s_bv2, s_gm1, s_bt1, s_gm2, s_bt2,
         s_lg1, s_lb1, s_lg1s, s_lb1s) = (
            s_bcat[:, i, :] for i in range(12))
        s_rcat = singles.tile([1, 4 * FD + H2], F32R, tag="rcat")
        nc.sync.dma_start(s_rcat[:], rcat)
        s_bc1 = s_rcat[:, 0:FD]
        s_bc2 = s_rcat[:, FD : 2 * FD]
        s_bg = s_rcat[:, 2 * FD : 3 * FD]
        s_bm1 = s_rcat[:, 3 * FD : 4 * FD]
        s_bm2 = s_rcat[:, 4 * FD : 4 * FD + H2]
        s_ones = singles.tile([1, P], F32R)
        nc.sync.dma_start(s_ones[:], onesr)
        s_id = singles.tile([P, P], F32R)
        nc.sync.dma_start(s_id[:], ident)

        s_wv1 = wload("wv1", wv1, 4, FD)
        s_wv2 = wload("wv2", wv2, 4, FD)
        s_wo1 = wload("wo1", wo1, 4, FD)
        s_wo2 = wload("wo2", wo2, 4, FD)
        prefetch(1)
        s_wg = wload("wg", wg, 8, FD)
        s_wm1 = wload("wm1", wm1, 8, H1)
        s_wm2 = wload("wm2", wm2, 4, H2)

        # integer constants for the rsqrt bit-trick seed
        s_i1 = singles.tile([P, 1], I32)
        nc.vector.memset(s_i1, 1)
        s_ineg1 = singles.tile([P, 1], I32)
        nc.vector.memset(s_ineg1, -1)
        s_magic = singles.tile([P, RT], I32)
        nc.vector.memset(s_magic, 0x5F3759DF + 1)

        if not ln2_identity:
            s_lg2b = singles.tile([P, H2], F32)
            nc.sync.dma_start(
                s_lg2b[:],
                bass.AP(tensor=lg2.tensor, offset=lg2.offset,
                        ap=[[0, P], *lg2.ap]),
            )
            s_lb2b = singles.tile([P, H2], F32)
            nc.sync.dma_start(
                s_lb2b[:],
                bass.AP(tensor=lb2.tensor, offset=lb2.offset,
                        ap=[[0, P], *lb2.ap]),
            )

        # ---- helpers ------------------------------------------------------
        def fm_layer(dst, w_sb, x_sb, ksub, bias_sb, engine):
            """features-major layer: dst[:, f, :] = w.T-block @ x + bias."""
            for f in range(4):
                pt = psA.tile([P, CH], F32, tag="psA")
                fs = slice(f * P, (f + 1) * P)
                for k in range(ksub):
                    nc.tensor.matmul(
                        pt[:],
                        w_sb[:, k, fs],
                        x_sb[:, k, :],
                        start=(k == 0),
                        stop=(k == ksub - 1),
                    )
                if engine == "act":
                    nc.scalar.activation(
                        dst[:, f, :], pt[:], Act.Identity,
                        bias=bias_sb[:, f : f + 1], scale=1.0,
                    )
                else:
                    nc.vector.tensor_scalar(
                        out=dst[:, f, :], in0=pt[:],
                        scalar1=bias_sb[:, f : f + 1], scalar2=None,
                        op0=Alu.add,
                    )

        def rm_psum(rt_, w_sb, act_sb, ksub, brow, nfree, resid_sb=None):
            """rows-major layer into PSUM: bias + sum_k act_k.T @ w_k (+resid.T)."""
            pt = ps.tile([P, nfree], F32, tag="ps")
            rs = slice(rt_ * P, (rt_ + 1) * P)
            nc.tensor.matmul(pt[:], s_ones[:], brow[:],
                             start=True, stop=False)
            nres = 4 if resid_sb is not None else 0
            for k in range(ksub):
                nc.tensor.matmul(
                    pt[:],
                    act_sb[:, k, rs],
                    w_sb[:, k, :],
                    start=False,
                    stop=(k == ksub - 1 and nres == 0),
                )
            for f in range(nres):
                fs = slice(f * P, (f + 1) * P)
                nc.tensor.matmul(
                    pt[:, fs].bitcast(F32R), resid_sb[:, f, rs], s_id[:],
                    is_transpose=True, start=False, stop=(f == nres - 1),
                )
            return pt

        def ln_rt(pt, dst):
            """Full per-rowtile LN chain: bn stats -> Newton rsqrt -> apply.

            Emitting the whole chain per rowtile keeps the DVE critical path
            short: rowtile i's chain runs while PE computes rowtile i+1."""
            st6 = stats.tile([P, 6], F32, tag="st6")
            nc.vector.bn_stats(st6[:], pt[:])
            mv = stats.tile([P, 2], F32, tag="mv")
            nc.vector.bn_aggr(mv[:], st6[:])
            vpe = stats.tile([P, 1], F32, tag="vpe")
            nc.vector.tensor_scalar(out=vpe[:], in0=mv[:, 1:2],
                                    scalar1=EPS, scalar2=None, op0=Alu.add)
            ji = stats.tile([P, 1], I32, tag="ji")
            nc.vector.tensor_scalar(out=ji[:], in0=vpe.bitcast(I32)[:],
                                    scalar1=s_i1[:], scalar2=None,
                                    op0=Alu.logical_shift_right)
            nc.vector.tensor_scalar(out=ji[:], in0=ji[:],
                                    scalar1=s_ineg1[:], scalar2=None,
                                    op0=Alu.bitwise_xor)
            nc.vector.tensor_tensor(ji[:], ji[:], s_magic[:, 0:1], Alu.add)
            rv = ji.bitcast(F32)  # Newton iterations run in-place on the seed
            tmp = stats.tile([P, 1], F32, tag="nrtmp")
            for _ in range(2):  # Newton: y *= 1.5 - 0.5*v*y^2
                nc.vector.tensor_mul(tmp[:], rv[:], rv[:])
                nc.vector.tensor_mul(tmp[:], tmp[:], vpe[:])
                nc.vector.tensor_scalar(out=tmp[:], in0=tmp[:],
                                        scalar1=-0.5, scalar2=1.5,
                                        op0=Alu.mult, op1=Alu.add)
                nc.vector.tensor_mul(rv[:], rv[:], tmp[:])
            nc.vector.tensor_scalar(
                out=dst, in0=pt[:], scalar1=mv[:, 0:1], scalar2=rv[:],
                op0=Alu.subtract, op1=Alu.mult,
            )

        def transpose_back(xh, f, gamma, beta, dst, func=Act.Identity):
            """xh [128, RT, 512] rows-major -> dst[:, :] features-major col f."""
            pc = ps.tile([P, CH], F32, tag="ps")
            fs = slice(f * P, (f + 1) * P)
            for t_ in range(RT):
                nc.tensor.matmul(
                    pc[:, t_ * P : (t_ + 1) * P].bitcast(F32R),
                    xh[:, t_, fs], s_id[:],
                    is_transpose=True, start=True, stop=True,
                )
            nc.scalar.activation(dst, pc[:], func,
                                 bias=beta[:, f : f + 1],
                                 scale=gamma[:, f : f + 1])
            return pc

        # ---- staged software pipeline over chunks -------------------------
        # Emission order interleaves chunks so PE always has matmul work
        # while DVE runs LayerNorm chains:
        #   A0 X0 A1 | G0 X1 L2+A2 T0 | G1 X2 L3+A3 T1 | G2 X3 T2 | G3 T3
        STAGES.clear()

        def mark(label):
            STAGES.append((int(nc.get_next_instruction_name()[2:]), label))

        S = [dict() for _ in range(NCH)]

        def stage_load(c):
            cs = slice(c * CH, (c + 1) * CH)
            if c < len(pref):
                S[c]["xt"], S[c]["xn"] = pref[c]
            else:
                xtc = iop.tile([P, 2, CH], F32R, tag="xtc")
                nc.sync.dma_start(xtc[:], xt3[:, :, cs])
                xnc = iop.tile([P, 2, CH], F32R, tag="xnc")
                nc.sync.dma_start(xnc[:], xn3[:, :, cs])
                S[c]["xt"], S[c]["xn"] = xtc, xnc

        def stage_A(c):
            mark(f"c{c}:A")
            s = S[c]
            s["t"] = acts.tile([P, 4, CH], F32R, tag="t_fm", name="t_fm")
            s["n"] = acts.tile([P, 4, CH], F32R, tag="n_fm", name="n_fm")
            s["v1"] = acts.tile([P, 4, CH], F32R, tag="v1_fm", name="v1_fm")
            s["v2"] = acts.tile([P, 4, CH], F32R, tag="v2_fm", name="v2_fm")
            fm_layer(s["t"], s_wtp, s["xt"], 2, s_btp, "act")
            fm_layer(s["n"], s_wnp, s["xn"], 2, s_bnp, "act")
            fm_layer(s["v1"], s_wv1, s["t"], 4, s_bv1, "act")
            fm_layer(s["v2"], s_wv2, s["n"], 4, s_bv2, "act")

        def stage_X(c):
            mark(f"c{c}:X")
            s = S[c]
            s["xh1"] = acts.tile([P, RT, FD], F32R, tag="xh1", name="xh1")
            s["xh2"] = acts.tile([P, RT, FD], F32R, tag="xh2", name="xh2")
            for t_ in range(RT):
                pt = rm_psum(t_, s_wo1, s["v1"], 4, s_bc1, FD,
                             resid_sb=s["n"])
                ln_rt(pt, s["xh1"][:, t_, :])
                pt = rm_psum(t_, s_wo2, s["v2"], 4, s_bc2, FD,
                             resid_sb=s["t"])
                ln_rt(pt, s["xh2"][:, t_, :])

        def stage_G(c):
            mark(f"c{c}:G")
            s = S[c]
            s["comb"] = acts.tile([P, 8, CH], F32R, tag="comb", name="comb")
            s["xh3"] = acts.tile([P, RT, H1], F32R, tag="xh3", name="xh3")
            gate_rm = outp.tile([P, RT, FD], F32, tag="gate_rm")
            for f in range(4):
                transpose_back(s["xh1"], f, s_gm1, s_bt1, s["comb"][:, f, :])
                transpose_back(s["xh2"], f, s_gm2, s_bt2,
                               s["comb"][:, 4 + f, :])
            for t_ in range(RT):
                pg = rm_psum(t_, s_wg, s["comb"], 8, s_bg, FD)
                nc.scalar.activation(gate_rm[:, t_, :], pg[:], Act.Sigmoid,
                                     bias=0.0, scale=1.0)
            nc.sync.dma_start(gate4[:, c, :, :], gate_rm[:])
            for t_ in range(RT):
                pt = rm_psum(t_, s_wm1, s["comb"], 8, s_bm1, H1)
                ln_rt(pt, s["xh3"][:, t_, :])

        def stage_T(c):
            mark(f"c{c}:T")
            s = S[c]
            s["h"] = acts.tile([P, 4, CH], F32R, tag="h_fm", name="h_fm")
            xh4 = acts.tile([P, RT, H2], F32, tag="xh4")
            fused_rm = outp.tile([P, RT, H2], F32, tag="fused_rm")
            for f in range(4):
                # z = xh*lg1+lb1 ; e = erf(z/sqrt2) ; h = z*(0.5+0.5e)
                pc = ps.tile([P, CH], F32, tag="ps")
                fs = slice(f * P, (f + 1) * P)
                for t_ in range(RT):
                    nc.tensor.matmul(
                        pc[:, t_ * P : (t_ + 1) * P].bitcast(F32R),
                        s["xh3"][:, t_, fs], s_id[:],
                        is_transpose=True, start=True, stop=True,
                    )
                e_t = zep.tile([P, CH], F32, tag="e")
                nc.scalar.activation(e_t[:], pc[:], Act.Erf,
                                     bias=s_lb1s[:, f : f + 1],
                                     scale=s_lg1s[:, f : f + 1])
                nc.gpsimd.tensor_scalar(out=e_t[:], in0=e_t[:],
                                        scalar1=0.5, scalar2=0.5,
                                        op0=Alu.mult, op1=Alu.add)
                if ln1_identity:
                    # z == pc (gamma=1, beta=0): multiply straight from PSUM
                    nc.vector.tensor_mul(s["h"][:, f, :], pc[:], e_t[:])
                else:
                    z_t = zep.tile([P, CH], F32, tag="z")
                    nc.scalar.activation(z_t[:], pc[:], Act.Identity,
                                         bias=s_lb1[:, f : f + 1],
                                         scale=s_lg1[:, f : f + 1])
                    nc.vector.tensor_mul(s["h"][:, f, :], z_t[:], e_t[:])
            for t_ in range(RT):
                pt = rm_psum(t_, s_wm2, s["h"], 4, s_bm2, H2)
                ln_rt(pt, xh4[:, t_, :])
            if not ln2_identity:
                for t_ in range(RT):
                    nc.gpsimd.tensor_mul(xh4[:, t_, :], xh4[:, t_, :],
                                         s_lg2b[:])
                    nc.gpsimd.tensor_add(xh4[:, t_, :], xh4[:, t_, :],
                                         s_lb2b[:])
            for t_ in range(RT):
                e2 = zep.tile([P, H2], F32, tag="e2")
                nc.scalar.activation(e2[:], xh4[:, t_, :], Act.Erf,
                                     bias=0.0, scale=INV_SQRT2)
                nc.gpsimd.tensor_scalar(out=e2[:], in0=e2[:],
                                        scalar1=0.5, scalar2=0.5,
                                        op0=Alu.mult, op1=Alu.add)
                nc.gpsimd.tensor_mul(fused_rm[:, t_, :], xh4[:, t_, :],
                                     e2[:])
            nc.sync.dma_start(fused4[:, c, :, :], fused_rm[:])
            s.clear()

        stage_load(0)
        stage_load(1)
        stage_A(0)
        stage_X(0)
        stage_A(1)
        for c in range(NCH):
            stage_G(c)
            if c + 1 < NCH:
                stage_X(c + 1)
            if c + 2 < NCH:
                stage_load(c + 2)
                stage_A(c + 2)
            stage_T(c)

    nc.compile()
    return nc


def _get_nc(ln1_identity: bool, ln2_identity: bool):
    key = ("nc", ln1_identity, ln2_identity)
    if key not in _CACHE:
        _CACHE[key] = _build_nc(ln1_identity, ln2_identity)
    return _CACHE[key]


def _make_in_maps(inp):
    f32 = np.float32

    def cT(a):  # transposed contiguous fp32
        return np.ascontiguousarray(np.asarray(a, f32).T)

    def c_(a):
        return np.ascontiguousarray(np.asarray(a, f32))

    bcat = np.stack([
        c_(inp["tp_b"]).reshape(4, P), c_(inp["np_b"]).reshape(4, P),
        c_(inp["a1_bv"]).reshape(4, P), c_(inp["a2_bv"]).reshape(4, P),
        c_(inp["n1_g"]).reshape(4, P), c_(inp["n1_b"]).reshape(4, P),
        c_(inp["n2_g"]).reshape(4, P), c_(inp["n2_b"]).reshape(4, P),
        c_(inp["ln1_g"]).reshape(4, P), c_(inp["ln1_b"]).reshape(4, P),
        (c_(inp["ln1_g"]) * f32(INV_SQRT2)).reshape(4, P),
        (c_(inp["ln1_b"]) * f32(INV_SQRT2)).reshape(4, P),
    ]).astype(f32)
    rcat = np.concatenate([
        c_(inp["a1_bo"]), c_(inp["a2_bo"]), c_(inp["g_b"]),
        c_(inp["m1_b"]), c_(inp["m2_b"]),
    ])[None, :].astype(f32)
    shared = {
        "wtp": cT(inp["tp_w"]), "wnp": cT(inp["np_w"]),
        "wv1": cT(inp["a1_wv"]), "wo1": cT(inp["a1_wo"]),
        "wv2": cT(inp["a2_wv"]), "wo2": cT(inp["a2_wo"]),
        "wg": cT(inp["g_w"]), "wm1": cT(inp["m1_w"]), "wm2": cT(inp["m2_w"]),
        "bcat": bcat, "rcat": rcat,
        "lg2": c_(inp["ln2_g"]), "lb2": c_(inp["ln2_b"]),
        "ident": np.eye(P, dtype=f32),
        "onesr": np.ones((1, P), f32),
    }
    xtT = cT(inp["text_emb"])   # [256, B]
    xnT = cT(inp["num_emb"])
    in_maps = []
    for core in range(NCORES):
        sl = slice(core * BC, (core + 1) * BC)
        m = dict(shared)
        m["xt"] = np.ascontiguousarray(xtT[:, sl])
        m["xn"] = np.ascontiguousarray(xnT[:, sl])
        in_maps.append(m)
    return in_maps


def kernel(
    text_emb, num_emb, tp_w, tp_b, np_w, np_b,
    a1_wq, a1_wk, a1_wv, a1_bq, a1_bk, a1_bv, a1_wo, a1_bo,
    a2_wq, a2_wk, a2_wv, a2_bq, a2_bk, a2_bv, a2_wo, a2_bo,
    n1_g, n1_b, n2_g, n2_b, g_w, g_b,
    m1_w, m1_b, ln1_g, ln1_b, m2_w, m2_b, ln2_g, ln2_b,
):
    from concourse.bass_utils import run_bass_kernel_spmd

    f32 = np.float32
    ln1_identity = bool(
        np.all(np.asarray(ln1_g) == 1.0) and np.all(np.asarray(ln1_b) == 0.0)
    )
    ln2_identity = bool(
        np.all(np.asarray(ln2_g) == 1.0) and np.all(np.asarray(ln2_b) == 0.0)
    )
    nc = _get_nc(ln1_identity, ln2_identity)
    in_maps = _make_in_maps(dict(
        text_emb=text_emb, num_emb=num_emb, tp_w=tp_w, tp_b=tp_b,
        np_w=np_w, np_b=np_b, a1_wv=a1_wv, a1_bv=a1_bv, a1_wo=a1_wo,
        a1_bo=a1_bo, a2_wv=a2_wv, a2_bv=a2_bv, a2_wo=a2_wo, a2_bo=a2_bo,
        n1_g=n1_g, n1_b=n1_b, n2_g=n2_g, n2_b=n2_b, g_w=g_w, g_b=g_b,
        m1_w=m1_w, m1_b=m1_b, ln1_g=ln1_g, ln1_b=ln1_b,
        m2_w=m2_w, m2_b=m2_b, ln2_g=ln2_g, ln2_b=ln2_b,
    ))

    res = run_bass_kernel_spmd(nc, in_maps, core_ids=list(range(NCORES)))
    fused = np.concatenate([r["fused"] for r in res.results], axis=0)
    gate = np.concatenate([r["gate"] for r in res.results], axis=0)
    attn = np.ones((B, 1, 1), f32)
    return fused, gate, attn, attn.copy()
